# revision 6
# baseline (speedup 1.0000x reference)
"""Trainium2 Bass kernel for the 2-layer GRU discriminator
(B=1024, T=63, F=257, H=512  ->  out [1024, 1]).

Strategy (pure data parallelism over batch, 8 cores x 128 batch each):
  - All weights/activations resident in SBUF; x streamed per timestep.
  - State kept as h [b=128 partitions, H free] in bf16; per-step PE
    transposes produce hT used as the matmul stationary operand, so gate
    matmuls run with the (static, SBUF-resident) weight matrices as the
    moving operand at N=512 free-dim, in bf16 (1 cycle/row vs 4 for fp32).
  - Gate preacts accumulate in PSUM fp32: x-contribution, hidden
    contribution and biases (via a ones-column appended to x, and K=1
    ones-row matmuls) all land in the same bank, so sigmoid/tanh read PSUM
    directly.
  - The entire MLP head collapses to out[b] = sum_t v[t]*(c_t . dnn_w) + c0
    (v = w3@w2@w1), accumulated across all 63 steps into one PSUM bank by
    M=1 matmuls against the per-step transposed state.
"""
import numpy as np
import ml_dtypes
from contextlib import ExitStack

import concourse.bass as bass
import concourse.tile as tile
from concourse import bacc, mybir
from concourse.bass_utils import run_bass_kernel_spmd

AF = mybir.ActivationFunctionType
OP = mybir.AluOpType
F32 = mybir.dt.float32
BF16 = mybir.dt.bfloat16
NPBF = ml_dtypes.bfloat16

B, T, F, H = 1024, 63, 257, 512
NCORES = 8
BC = B // NCORES          # 128 batch per core
G3 = 3 * H                # 1536
FP = 384                  # padded feature dim (257 data + 1 ones + zeros)
NF = FP // 128            # 3 feature chunks
NK = H // 128             # 4 hidden chunks


def _build_module():
    nc = bacc.Bacc("TRN2", target_bir_lowering=False, debug=False)

    xT_d = nc.dram_tensor("xT", [T, 128, NF * BC], BF16, kind="ExternalInput").ap()
    wih0_d = nc.dram_tensor("wih0", [NF, 128, G3], BF16, kind="ExternalInput").ap()
    whh0_d = nc.dram_tensor("whh0", [NK, 128, G3], BF16, kind="ExternalInput").ap()
    wih1_d = nc.dram_tensor("wih1", [NK, 128, G3], BF16, kind="ExternalInput").ap()
    whh1_d = nc.dram_tensor("whh1", [NK, 128, G3], BF16, kind="ExternalInput").ap()
    brow_d = nc.dram_tensor("brow", [1, 5 * H], BF16, kind="ExternalInput").ap()
    ones_d = nc.dram_tensor("ones", [1, BC], BF16, kind="ExternalInput").ap()
    iden_d = nc.dram_tensor("iden", [128, 128], BF16, kind="ExternalInput").ap()
    dnsc_d = nc.dram_tensor("dnsc", [NK, 128, T], BF16, kind="ExternalInput").ap()
    out_d = nc.dram_tensor("out", [1, BC], F32, kind="ExternalOutput").ap()

    with tile.TileContext(nc) as tc, ExitStack() as ctx:
        wp = ctx.enter_context(tc.tile_pool(name="wp", bufs=1, space="SBUF"))
        xp = ctx.enter_context(tc.tile_pool(name="xp", bufs=4, space="SBUF"))
        sp = ctx.enter_context(tc.tile_pool(name="sp", bufs=2, space="SBUF"))
        pg = ctx.enter_context(tc.tile_pool(name="pg", bufs=5, space="PSUM"))
        pt = ctx.enter_context(tc.tile_pool(name="pt", bufs=2, space="PSUM"))
        ph = ctx.enter_context(tc.tile_pool(name="ph", bufs=1, space="PSUM"))

        # --- resident weights ---
        wih0 = [wp.tile_from(wih0_d[j], name=f"wih0_{j}") for j in range(NF)]
        whh0 = [wp.tile_from(whh0_d[k], name=f"whh0_{k}") for k in range(NK)]
        brow = wp.tile_from(brow_d, name="brow")
        ones = wp.tile_from(ones_d, name="ones")
        wih1 = [wp.tile_from(wih1_d[k], name=f"wih1_{k}") for k in range(NK)]
        whh1 = [wp.tile_from(whh1_d[k], name=f"whh1_{k}") for k in range(NK)]
        iden = wp.tile_from(iden_d, name="iden")
        dnsc = [wp.tile_from(dnsc_d[k], name=f"dnsc_{k}") for k in range(NK)]

        bhh0n = brow[:, 0:H]
        b1r = brow[:, H:2 * H]
        b1z = brow[:, 2 * H:3 * H]
        bhh1n = brow[:, 3 * H:4 * H]
        bih1n = brow[:, 4 * H:5 * H]

        head = ph.tile([1, BC], F32, name="head", tag="head", bufs=1)

        aT = None
        cT = None
        a_prev = None
        c_prev = None

        def gru_gates(pr, pz, phn, pxn, prev, nm):
            """PSUM preacts -> new state [BC, H] bf16 in SBUF."""
            r = sp.tile([BC, H], BF16, name=f"r_{nm}", tag=f"r_{nm}")
            z = sp.tile([BC, H], BF16, name=f"z_{nm}", tag=f"z_{nm}")
            nn = sp.tile([BC, H], BF16, name=f"n_{nm}", tag=f"n_{nm}")
            t3 = sp.tile([BC, H], BF16, name=f"t3_{nm}", tag=f"t3_{nm}")
            t4 = sp.tile([BC, H], F32, name=f"t4_{nm}", tag=f"t4_{nm}")
            hnew = sp.tile([BC, H], BF16, name=f"h_{nm}", tag=f"h_{nm}")
            nc.scalar.activation(out=r, in_=pr, func=AF.Sigmoid)
            nc.scalar.activation(out=z, in_=pz, func=AF.Sigmoid)
            nc.vector.tensor_tensor(out=t3, in0=r, in1=phn, op=OP.mult)
            nc.vector.tensor_tensor(out=t4, in0=t3, in1=pxn, op=OP.add)
            nc.scalar.activation(out=nn, in_=t4, func=AF.Tanh)
            if prev is None:
                # h' = n - z*n
                nc.vector.tensor_tensor(out=t3, in0=z, in1=nn, op=OP.mult)
                nc.vector.tensor_tensor(out=hnew, in0=nn, in1=t3, op=OP.subtract)
            else:
                # h' = n + z*(h - n)
                t5 = sp.tile([BC, H], BF16, name=f"t5_{nm}", tag=f"t5_{nm}")
                t6 = sp.tile([BC, H], BF16, name=f"t6_{nm}", tag=f"t6_{nm}")
                nc.vector.tensor_tensor(out=t5, in0=prev, in1=nn, op=OP.subtract)
                nc.vector.tensor_tensor(out=t6, in0=z, in1=t5, op=OP.mult)
                nc.vector.tensor_tensor(out=hnew, in0=nn, in1=t6, op=OP.add)
            return hnew

        def transpose_state(h, nm):
            """[BC, H] SBUF bf16 -> [128, H] SBUF bf16 holding hT chunks."""
            ptr = pt.tile([128, H], BF16, name=f"ptr_{nm}", tag="tr")
            for k in range(NK):
                nc.tensor.transpose(
                    out=ptr[:, k * 128:(k + 1) * 128],
                    in_=h[:, k * 128:(k + 1) * 128],
                    identity=iden,
                )
            hT = sp.tile([128, H], BF16, name=f"hT_{nm}", tag=f"hT_{nm}")
            nc.scalar.activation(out=hT, in_=ptr, func=AF.Copy)
            return hT

        for t in range(T):
            # ---- stream x_t (pre-transposed on host) ----
            xt = xp.tile([128, NF * BC], BF16, name="xt", tag="xt")
            nc.sync.dma_start(out=xt, in_=xT_d[t])

            # ---- layer 0 matmuls ----
            pr = pg.tile([BC, H], F32, name="pr0", tag="g")
            pz = pg.tile([BC, H], F32, name="pz0", tag="g")
            phn = pg.tile([BC, H], F32, name="phn0", tag="g")
            pxn = pg.tile([BC, H], F32, name="pxn0", tag="g")
            for j in range(NF):
                xk = xt[:, j * BC:(j + 1) * BC]
                nc.tensor.matmul(pr, xk, wih0[j][:, 0:H],
                                 start=(j == 0), stop=(j == NF - 1 and t == 0))
                nc.tensor.matmul(pz, xk, wih0[j][:, H:2 * H],
                                 start=(j == 0), stop=(j == NF - 1 and t == 0))
                nc.tensor.matmul(pxn, xk, wih0[j][:, 2 * H:G3],
                                 start=(j == 0), stop=(j == NF - 1))
            nc.tensor.matmul(phn, ones, bhh0n, start=True, stop=(t == 0))
            if t > 0:
                for k in range(NK):
                    ak = aT[:, k * 128:(k + 1) * 128]
                    nc.tensor.matmul(pr, ak, whh0[k][:, 0:H],
                                     start=False, stop=(k == NK - 1))
                    nc.tensor.matmul(pz, ak, whh0[k][:, H:2 * H],
                                     start=False, stop=(k == NK - 1))
                    nc.tensor.matmul(phn, ak, whh0[k][:, 2 * H:G3],
                                     start=False, stop=(k == NK - 1))

            a_new = gru_gates(pr, pz, phn, pxn, a_prev, "a")
            aT = transpose_state(a_new, "a")
            a_prev = a_new

            # ---- layer 1 matmuls ----
            pr1 = pg.tile([BC, H], F32, name="pr1", tag="g")
            pz1 = pg.tile([BC, H], F32, name="pz1", tag="g")
            phn1 = pg.tile([BC, H], F32, name="phn1", tag="g")
            pxn1 = pg.tile([BC, H], F32, name="pxn1", tag="g")
            nc.tensor.matmul(pr1, ones, b1r, start=True, stop=False)
            nc.tensor.matmul(pz1, ones, b1z, start=True, stop=False)
            nc.tensor.matmul(phn1, ones, bhh1n, start=True, stop=(t == 0))
            nc.tensor.matmul(pxn1, ones, bih1n, start=True, stop=False)
            for k in range(NK):
                ak = aT[:, k * 128:(k + 1) * 128]
                nc.tensor.matmul(pr1, ak, wih1[k][:, 0:H],
                                 start=False, stop=(k == NK - 1 and t == 0))
                nc.tensor.matmul(pz1, ak, wih1[k][:, H:2 * H],
                                 start=False, stop=(k == NK - 1 and t == 0))
                nc.tensor.matmul(pxn1, ak, wih1[k][:, 2 * H:G3],
                                 start=False, stop=(k == NK - 1))
            if t > 0:
                for k in range(NK):
                    ck = cT[:, k * 128:(k + 1) * 128]
                    nc.tensor.matmul(pr1, ck, whh1[k][:, 0:H],
                                     start=False, stop=(k == NK - 1))
                    nc.tensor.matmul(pz1, ck, whh1[k][:, H:2 * H],
                                     start=False, stop=(k == NK - 1))
                    nc.tensor.matmul(phn1, ck, whh1[k][:, 2 * H:G3],
                                     start=False, stop=(k == NK - 1))

            c_new = gru_gates(pr1, pz1, phn1, pxn1, c_prev, "c")
            cT = transpose_state(c_new, "c")
            c_prev = c_new

            # ---- head accumulation ----
            for k in range(NK):
                nc.tensor.matmul(head, dnsc[k][:, t:t + 1],
                                 cT[:, k * 128:(k + 1) * 128],
                                 start=(t == 0 and k == 0),
                                 stop=(t == T - 1 and k == NK - 1))

        out_sb = sp.tile([1, BC], F32, name="out_sb", tag="out_sb")
        nc.scalar.activation(out=out_sb, in_=head, func=AF.Copy)
        nc.sync.dma_start(out=out_d, in_=out_sb)

    # legalize sem waits (>=2 waits per matmul is a codegen error) etc.
    nc.compile()
    return nc


def host_prep(inputs):
    f32 = np.float32
    x = np.asarray(inputs["x"], f32)
    w_ih0, w_hh0 = np.asarray(inputs["w_ih0"], f32), np.asarray(inputs["w_hh0"], f32)
    b_ih0, b_hh0 = np.asarray(inputs["b_ih0"], f32), np.asarray(inputs["b_hh0"], f32)
    w_ih1, w_hh1 = np.asarray(inputs["w_ih1"], f32), np.asarray(inputs["w_hh1"], f32)
    b_ih1, b_hh1 = np.asarray(inputs["b_ih1"], f32), np.asarray(inputs["b_hh1"], f32)
    dnn_w, dnn_b = np.asarray(inputs["dnn_w"], f32), np.asarray(inputs["dnn_b"], f32)
    w1, b1 = np.asarray(inputs["w1"], f32), np.asarray(inputs["b1"], f32)
    w2, b2 = np.asarray(inputs["w2"], f32), np.asarray(inputs["b2"], f32)
    w3, b3 = np.asarray(inputs["w3"], f32), np.asarray(inputs["b3"], f32)

    wih0 = np.zeros((FP, G3), f32)
    wih0[:F] = w_ih0.T
    wih0[F] = np.concatenate([(b_ih0 + b_hh0)[:2 * H], b_ih0[2 * H:]])
    wih0 = np.ascontiguousarray(wih0.reshape(NF, 128, G3)).astype(NPBF)

    whh0 = np.ascontiguousarray(w_hh0.T.reshape(NK, 128, G3)).astype(NPBF)
    wih1 = np.ascontiguousarray(w_ih1.T.reshape(NK, 128, G3)).astype(NPBF)
    whh1 = np.ascontiguousarray(w_hh1.T.reshape(NK, 128, G3)).astype(NPBF)

    b1g = b_ih1 + b_hh1
    brow = np.concatenate(
        [b_hh0[2 * H:], b1g[:H], b1g[H:2 * H], b_hh1[2 * H:], b_ih1[2 * H:]]
    ).reshape(1, 5 * H).astype(NPBF)

    v = (w3 @ w2 @ w1)[0]
    dnsc = np.ascontiguousarray(
        (dnn_w[0][:, None] * v[None, :]).reshape(NK, 128, T)).astype(NPBF)
    c_all = float(v.sum() * dnn_b[0] + (w3 @ w2 @ b1)[0] + (w3 @ b2)[0] + b3[0])

    shared = dict(
        wih0=wih0, whh0=whh0, wih1=wih1, whh1=whh1, brow=brow,
        ones=np.ones((1, BC), NPBF), iden=np.eye(128, dtype=NPBF), dnsc=dnsc)

    xcores = []
    for c in range(NCORES):
        xc = x[c * BC:(c + 1) * BC]
        xpad = np.zeros((BC, T, FP), f32)
        xpad[:, :, :F] = xc
        xpad[:, :, F] = 1.0
        xT = xpad.reshape(BC, T, NF, 128).transpose(1, 3, 2, 0).reshape(T, 128, NF * BC)
        xcores.append(np.ascontiguousarray(xT).astype(NPBF))
    return shared, xcores, c_all


_CACHED = {}


def _get_module():
    if "nc" not in _CACHED:
        _CACHED["nc"] = _build_module()
    return _CACHED["nc"]


def kernel(**inputs) -> np.ndarray:
    shared, xcores, c_all = host_prep(inputs)
    nc = _get_module()
    in_maps = [{**shared, "xT": xcores[c]} for c in range(NCORES)]
    res = run_bass_kernel_spmd(nc, in_maps, core_ids=list(range(NCORES)))
    outs = [res.results[c]["out"].reshape(BC) for c in range(NCORES)]
    out = np.concatenate(outs).astype(np.float32) + np.float32(c_all)
    return out.reshape(B, 1)


# revision 10
# speedup vs baseline: 1.2123x; 1.2123x over previous
"""Trainium2 Bass kernel for the 2-layer GRU discriminator
(B=1024, T=63, F=257, H=512  ->  out [1024, 1]).

Strategy (pure data parallelism over batch, 8 cores x 128 batch each):
  - All weights/activations resident in SBUF; x streamed per timestep.
  - State kept as h [b=128 partitions, H free] in bf16; per-step PE
    transposes produce hT (cast to fp8) used as the matmul stationary
    operand, so gate matmuls run with the (static, SBUF-resident) weight
    matrices as the moving operand at N=512 free-dim.
  - Gate matmuls are fp8e4m3 with perf_mode=DoubleRow: K-chunk pairs are
    packed [128, 2, dim] so each matmul contracts 256 rows (2 fp8
    weights/cell), halving PE streaming time. Weights are pre-scaled by
    WSCALE=16 to stay in fp8's normal range; sigmoids/tanh descale via the
    free `scale=` affine of the ACT instruction.
  - Gate preacts accumulate in PSUM fp32: x-contribution, hidden
    contribution and biases (via a ones-column appended to x, and K=1
    ones-row matmuls) all land in the same bank, so sigmoid/tanh read PSUM
    directly.
  - The entire MLP head collapses to out[b] = sum_t v[t]*(c_t . dnn_w) + c0
    (v = w3@w2@w1), accumulated across all 63 steps into one PSUM bank by
    M=1 matmuls against the per-step transposed state.
"""
import numpy as np
import ml_dtypes
from contextlib import ExitStack

import concourse.bass as bass
import concourse.tile as tile
from concourse import bacc, mybir
from concourse.bass_utils import run_bass_kernel_spmd

AF = mybir.ActivationFunctionType
OP = mybir.AluOpType
PM = mybir.MatmulPerfMode
F32 = mybir.dt.float32
BF16 = mybir.dt.bfloat16
FP8 = mybir.dt.float8e4
NPBF = ml_dtypes.bfloat16
NPF8 = ml_dtypes.float8_e4m3

B, T, F, H = 1024, 63, 257, 512
NCORES = 8
BC = B // NCORES          # 128 batch per core
G3 = 3 * H                # 1536
FP = 512                  # padded feature dim (257 data + 1 ones + zeros)
NF = FP // 128            # 4 feature chunks
NK = H // 128             # 4 hidden chunks
NFQ = NF // 2             # 2 feature chunk-pairs (DoubleRow)
NKQ = NK // 2             # 2 hidden chunk-pairs
WSCALE = 16.0             # fp8 weight pre-scale (descaled in sigmoid/tanh)
DSCALE = 4096.0           # head dnn-weight pre-scale (descaled in out copy)


def _dr(ap):
    """[128, 2*X] slice -> [128, 2, X] chunk-pair AP for DoubleRow."""
    return ap.rearrange("p (i b) -> p i b", i=2)


def _build_module():
    nc = bacc.Bacc("TRN2", target_bir_lowering=False, debug=False)

    xT_d = nc.dram_tensor("xT", [T, 128, NF * BC], FP8, kind="ExternalInput").ap()
    wih0_d = nc.dram_tensor("wih0", [NFQ, 128, 2 * G3], FP8, kind="ExternalInput").ap()
    whh0_d = nc.dram_tensor("whh0", [NKQ, 128, 2 * G3], FP8, kind="ExternalInput").ap()
    wih1_d = nc.dram_tensor("wih1", [NKQ, 128, 2 * G3], FP8, kind="ExternalInput").ap()
    whh1_d = nc.dram_tensor("whh1", [NKQ, 128, 2 * G3], FP8, kind="ExternalInput").ap()
    brow_d = nc.dram_tensor("brow", [1, 5 * H], BF16, kind="ExternalInput").ap()
    ones_d = nc.dram_tensor("ones", [1, BC], BF16, kind="ExternalInput").ap()
    iden_d = nc.dram_tensor("iden", [128, 128], BF16, kind="ExternalInput").ap()
    dnsc_d = nc.dram_tensor("dnsc", [NK, 128, T], FP8, kind="ExternalInput").ap()
    out_d = nc.dram_tensor("out", [1, BC], F32, kind="ExternalOutput").ap()

    with tile.TileContext(nc) as tc, ExitStack() as ctx:
        wp = ctx.enter_context(tc.tile_pool(name="wp", bufs=1, space="SBUF"))
        xp = ctx.enter_context(tc.tile_pool(name="xp", bufs=4, space="SBUF"))
        sp = ctx.enter_context(tc.tile_pool(name="sp", bufs=2, space="SBUF"))
        pg = ctx.enter_context(tc.tile_pool(name="pg", bufs=6, space="PSUM"))
        pt = ctx.enter_context(tc.tile_pool(name="pt", bufs=1, space="PSUM"))
        ph = ctx.enter_context(tc.tile_pool(name="ph", bufs=1, space="PSUM"))

        # --- resident weights (chunk-pair packed for DoubleRow) ---
        wih0 = [wp.tile_from(wih0_d[q], name=f"wih0_{q}") for q in range(NFQ)]
        whh0 = [wp.tile_from(whh0_d[q], name=f"whh0_{q}") for q in range(NKQ)]
        brow = wp.tile_from(brow_d, name="brow")
        ones = wp.tile_from(ones_d, name="ones")
        wih1 = [wp.tile_from(wih1_d[q], name=f"wih1_{q}") for q in range(NKQ)]
        whh1 = [wp.tile_from(whh1_d[q], name=f"whh1_{q}") for q in range(NKQ)]
        iden = wp.tile_from(iden_d, name="iden")
        dnsc = [wp.tile_from(dnsc_d[k], name=f"dnsc_{k}") for k in range(NK)]

        def wslice(wtile, g0, g1):
            """[128, 2*G3] pair tile -> [128, 2, g1-g0] moving operand."""
            return wtile.rearrange("p (i g) -> p i g", i=2)[:, :, g0:g1]

        bhh0n = brow[:, 0:H]
        b1r = brow[:, H:2 * H]
        b1z = brow[:, 2 * H:3 * H]
        bhh1n = brow[:, 3 * H:4 * H]
        bih1n = brow[:, 4 * H:5 * H]

        head = ph.tile([1, BC], F32, name="head", tag="head", bufs=1)

        aT = None
        cT = None
        a_prev = None
        c_prev = None

        def gru_gates(pr, pz, phn, pxn, prev, nm):
            """PSUM preacts (x WSCALE) -> new state [BC, H] bf16 in SBUF."""
            r = sp.tile([BC, H], BF16, name=f"r_{nm}", tag=f"r_{nm}")
            z = sp.tile([BC, H], BF16, name=f"z_{nm}", tag=f"z_{nm}")
            nn = sp.tile([BC, H], BF16, name=f"n_{nm}", tag=f"n_{nm}")
            t3 = sp.tile([BC, H], BF16, name=f"t3_{nm}", tag=f"t3_{nm}")
            t4 = sp.tile([BC, H], F32, name=f"t4_{nm}", tag=f"t4_{nm}")
            hnew = sp.tile([BC, H], BF16, name=f"h_{nm}", tag=f"h_{nm}")
            nc.scalar.activation(out=r, in_=pr, func=AF.Sigmoid, scale=1.0 / WSCALE)
            nc.scalar.activation(out=z, in_=pz, func=AF.Sigmoid, scale=1.0 / WSCALE)
            nc.vector.tensor_tensor(out=t3, in0=r, in1=phn, op=OP.mult)
            nc.vector.tensor_tensor(out=t4, in0=t3, in1=pxn, op=OP.add)
            nc.scalar.activation(out=nn, in_=t4, func=AF.Tanh, scale=1.0 / WSCALE)
            if prev is None:
                # h' = n - z*n
                nc.vector.tensor_tensor(out=t3, in0=z, in1=nn, op=OP.mult)
                nc.vector.tensor_tensor(out=hnew, in0=nn, in1=t3, op=OP.subtract)
            else:
                # h' = n + z*(h - n)
                t5 = sp.tile([BC, H], BF16, name=f"t5_{nm}", tag=f"t5_{nm}")
                t6 = sp.tile([BC, H], BF16, name=f"t6_{nm}", tag=f"t6_{nm}")
                nc.vector.tensor_tensor(out=t5, in0=prev, in1=nn, op=OP.subtract)
                nc.vector.tensor_tensor(out=t6, in0=z, in1=t5, op=OP.mult)
                nc.vector.tensor_tensor(out=hnew, in0=nn, in1=t6, op=OP.add)
            return hnew

        def transpose_state(h, nm):
            """[BC, H] SBUF bf16 -> [128, H] SBUF fp8 holding hT chunks."""
            ptr = pt.tile([128, H], BF16, name=f"ptr_{nm}", tag="tr")
            for k in range(NK):
                nc.tensor.transpose(
                    out=ptr[:, k * 128:(k + 1) * 128],
                    in_=h[:, k * 128:(k + 1) * 128],
                    identity=iden,
                )
            hT = sp.tile([128, H], FP8, name=f"hT_{nm}", tag=f"hT_{nm}")
            nc.vector.tensor_copy(out=hT, in_=ptr)
            return hT

        for t in range(T):
            # ---- stream x_t (pre-transposed, padded, fp8 on host) ----
            xt = xp.tile([128, NF * BC], FP8, name="xt", tag="xt")
            nc.sync.dma_start(out=xt, in_=xT_d[t])

            # ---- layer 0 matmuls (DoubleRow fp8, K=256 per mm) ----
            pr = pg.tile([BC, H], F32, name="pr0", tag="g")
            pz = pg.tile([BC, H], F32, name="pz0", tag="g")
            phn = pg.tile([BC, H], F32, name="phn0", tag="g")
            pxn = pg.tile([BC, H], F32, name="pxn0", tag="g")
            for q in range(NFQ):
                xq = _dr(xt[:, 2 * q * BC:(2 * q + 2) * BC])
                nc.tensor.matmul(pr, xq, wslice(wih0[q], 0, H),
                                 start=(q == 0), stop=(q == NFQ - 1 and t == 0),
                                 perf_mode=PM.DoubleRow)
                nc.tensor.matmul(pz, xq, wslice(wih0[q], H, 2 * H),
                                 start=(q == 0), stop=(q == NFQ - 1 and t == 0),
                                 perf_mode=PM.DoubleRow)
                nc.tensor.matmul(pxn, xq, wslice(wih0[q], 2 * H, G3),
                                 start=(q == 0), stop=(q == NFQ - 1),
                                 perf_mode=PM.DoubleRow)
            nc.tensor.matmul(phn, ones, bhh0n, start=True, stop=(t == 0))
            if t > 0:
                for q in range(NKQ):
                    aq = _dr(aT[:, 2 * q * 128:(2 * q + 2) * 128])
                    nc.tensor.matmul(pr, aq, wslice(whh0[q], 0, H),
                                     start=False, stop=(q == NKQ - 1),
                                     perf_mode=PM.DoubleRow)
                    nc.tensor.matmul(pz, aq, wslice(whh0[q], H, 2 * H),
                                     start=False, stop=(q == NKQ - 1),
                                     perf_mode=PM.DoubleRow)
                    nc.tensor.matmul(phn, aq, wslice(whh0[q], 2 * H, G3),
                                     start=False, stop=(q == NKQ - 1),
                                     perf_mode=PM.DoubleRow)

            a_new = gru_gates(pr, pz, phn, pxn, a_prev, "a")
            aT = transpose_state(a_new, "a")
            a_prev = a_new

            # ---- layer 1 matmuls ----
            pr1 = pg.tile([BC, H], F32, name="pr1", tag="g")
            pz1 = pg.tile([BC, H], F32, name="pz1", tag="g")
            phn1 = pg.tile([BC, H], F32, name="phn1", tag="g")
            pxn1 = pg.tile([BC, H], F32, name="pxn1", tag="g")
            nc.tensor.matmul(pr1, ones, b1r, start=True, stop=False)
            nc.tensor.matmul(pz1, ones, b1z, start=True, stop=False)
            nc.tensor.matmul(phn1, ones, bhh1n, start=True, stop=(t == 0))
            nc.tensor.matmul(pxn1, ones, bih1n, start=True, stop=False)
            # cT-dependent matmuls first: cT(t-1) is ready at step start,
            # aT(t) only mid-step
            if t > 0:
                for q in range(NKQ):
                    cq = _dr(cT[:, 2 * q * 128:(2 * q + 2) * 128])
                    nc.tensor.matmul(pr1, cq, wslice(whh1[q], 0, H),
                                     start=False, stop=False,
                                     perf_mode=PM.DoubleRow)
                    nc.tensor.matmul(pz1, cq, wslice(whh1[q], H, 2 * H),
                                     start=False, stop=False,
                                     perf_mode=PM.DoubleRow)
                    nc.tensor.matmul(phn1, cq, wslice(whh1[q], 2 * H, G3),
                                     start=False, stop=(q == NKQ - 1),
                                     perf_mode=PM.DoubleRow)
            for q in range(NKQ):
                aq = _dr(aT[:, 2 * q * 128:(2 * q + 2) * 128])
                nc.tensor.matmul(pr1, aq, wslice(wih1[q], 0, H),
                                 start=False, stop=(q == NKQ - 1),
                                 perf_mode=PM.DoubleRow)
                nc.tensor.matmul(pz1, aq, wslice(wih1[q], H, 2 * H),
                                 start=False, stop=(q == NKQ - 1),
                                 perf_mode=PM.DoubleRow)
                nc.tensor.matmul(pxn1, aq, wslice(wih1[q], 2 * H, G3),
                                 start=False, stop=(q == NKQ - 1),
                                 perf_mode=PM.DoubleRow)

            c_new = gru_gates(pr1, pz1, phn1, pxn1, c_prev, "c")
            cT = transpose_state(c_new, "c")
            c_prev = c_new

            # ---- head accumulation (plain fp8 matmuls, M=1) ----
            for k in range(NK):
                nc.tensor.matmul(head, dnsc[k][:, t:t + 1],
                                 cT[:, k * 128:(k + 1) * 128],
                                 start=(t == 0 and k == 0),
                                 stop=(t == T - 1 and k == NK - 1))

        out_sb = sp.tile([1, BC], F32, name="out_sb", tag="out_sb")
        nc.scalar.activation(out=out_sb, in_=head, func=AF.Copy, scale=1.0 / DSCALE)
        nc.sync.dma_start(out=out_d, in_=out_sb)

    # legalize sem waits (>=2 waits per matmul is a codegen error) etc.
    nc.compile()
    return nc


def _pack_pairs(wt):
    """[512, G3] (contraction-major) -> [NQ, 128, 2*G3] chunk-pair tiles:
    out[q][p, i*G3+g] = wt[(2q+i)*128 + p, g]"""
    nq = wt.shape[0] // 256
    return np.ascontiguousarray(
        wt.reshape(nq, 2, 128, -1).transpose(0, 2, 1, 3).reshape(nq, 128, -1))


def host_prep(inputs):
    f32 = np.float32
    x = np.asarray(inputs["x"], f32)
    w_ih0, w_hh0 = np.asarray(inputs["w_ih0"], f32), np.asarray(inputs["w_hh0"], f32)
    b_ih0, b_hh0 = np.asarray(inputs["b_ih0"], f32), np.asarray(inputs["b_hh0"], f32)
    w_ih1, w_hh1 = np.asarray(inputs["w_ih1"], f32), np.asarray(inputs["w_hh1"], f32)
    b_ih1, b_hh1 = np.asarray(inputs["b_ih1"], f32), np.asarray(inputs["b_hh1"], f32)
    dnn_w, dnn_b = np.asarray(inputs["dnn_w"], f32), np.asarray(inputs["dnn_b"], f32)
    w1, b1 = np.asarray(inputs["w1"], f32), np.asarray(inputs["b1"], f32)
    w2, b2 = np.asarray(inputs["w2"], f32), np.asarray(inputs["b2"], f32)
    w3, b3 = np.asarray(inputs["w3"], f32), np.asarray(inputs["b3"], f32)

    wih0 = np.zeros((FP, G3), f32)
    wih0[:F] = w_ih0.T
    wih0[F] = np.concatenate([(b_ih0 + b_hh0)[:2 * H], b_ih0[2 * H:]])
    wih0 = _pack_pairs(wih0 * WSCALE).astype(NPF8)
    whh0 = _pack_pairs(w_hh0.T * WSCALE).astype(NPF8)
    wih1 = _pack_pairs(w_ih1.T * WSCALE).astype(NPF8)
    whh1 = _pack_pairs(w_hh1.T * WSCALE).astype(NPF8)

    b1g = b_ih1 + b_hh1
    brow = (np.concatenate(
        [b_hh0[2 * H:], b1g[:H], b1g[H:2 * H], b_hh1[2 * H:], b_ih1[2 * H:]]
    ).reshape(1, 5 * H) * WSCALE).astype(NPBF)

    v = (w3 @ w2 @ w1)[0]
    dnsc = np.ascontiguousarray(
        (dnn_w[0][:, None] * v[None, :] * DSCALE).reshape(NK, 128, T)).astype(NPF8)
    c_all = float(v.sum() * dnn_b[0] + (w3 @ w2 @ b1)[0] + (w3 @ b2)[0] + b3[0])

    shared = dict(
        wih0=wih0, whh0=whh0, wih1=wih1, whh1=whh1, brow=brow,
        ones=np.ones((1, BC), NPBF), iden=np.eye(128, dtype=NPBF), dnsc=dnsc)

    xcores = []
    for c in range(NCORES):
        xc = x[c * BC:(c + 1) * BC]
        xpad = np.zeros((BC, T, FP), f32)
        xpad[:, :, :F] = xc
        xpad[:, :, F] = 1.0
        xT = xpad.reshape(BC, T, NF, 128).transpose(1, 3, 2, 0).reshape(T, 128, NF * BC)
        xcores.append(np.ascontiguousarray(xT).astype(NPF8))
    return shared, xcores, c_all


_CACHED = {}


def _get_module():
    if "nc" not in _CACHED:
        _CACHED["nc"] = _build_module()
    return _CACHED["nc"]


def kernel(**inputs) -> np.ndarray:
    shared, xcores, c_all = host_prep(inputs)
    nc = _get_module()
    in_maps = [{**shared, "xT": xcores[c]} for c in range(NCORES)]
    res = run_bass_kernel_spmd(nc, in_maps, core_ids=list(range(NCORES)))
    outs = [res.results[c]["out"].reshape(BC) for c in range(NCORES)]
    out = np.concatenate(outs).astype(np.float32) + np.float32(c_all)
    return out.reshape(B, 1)
